# revision 12
# baseline (speedup 1.0000x reference)
"""Trainium2 Bass kernel for nn_ComputeCorr (retrieval_knn).

Math (per batch pair b, D=64 features):
  d[n,m] = ||sf[n]-tf[m]||^2,  sf = src_f[b].T, tf = tgt_f[b].T
  src_corr[b] = softmax_m(-d) @ tgt[b];  tgt_corr[b] = softmax_n(-d.T) @ src[b]

Restructure (per side, shown for src_corr):
  softmax_m(-d)[n,:] @ tgt = (sum_m U[m,n] * [tgt|1][m,:])[:3] / (...)[3]
  U[m,n] = exp(c0 - d[n,m]) computed directly in [m(part), n(free)] layout.
  The whole exponent (2*ab - aa[n] - bb[m] + c0)/2 comes from ONE fp16
  matmul with K-packed augmentation rows (K = 64 + 3):
    lhsT = [fp16(L); 1; bias_hi; bias_lo]   (bias = (c0 - |l_m|^2)/2)
    rhs  = [fp16(R); -|r_n|^2/2; 1; 1]
  so exponent == c0 - d <= c0: no max pass, no overflow, and the bf16/fp16
  rounding of the -|r_n|^2/2 row is a per-column shift that cancels in the
  softmax normalization.  ScalarE does a bias-free exp (scale=2.0) from
  PSUM over 1024-wide tiles, emitting U in bf16; one bf16 matmul against
  [tgt|1] accumulates numerator+denominator in PSUM over all 32 m-chunks
  (the stationary [tgt|1] slice is widened to 128 columns so FWL kicks in;
  output rows 4..127 are garbage we never read).  Epilogue: tiny PE
  transpose to [n, 4], reciprocal + multiply on DVE, DMA out.

Sharding: 8 cores = 4 batches x 2 halves; core c = batch c//2, rows
[h*2048,(h+1)*2048) of BOTH outputs (h=c%2). The [4096 x 2048] score
block per side is never materialized in DRAM.
"""

import os
import sys

import numpy as np

for _p in ("/opt/trn_rl_repo", "/root/.axon_site/_ro/trn_rl_repo"):
    if os.path.isdir(_p) and _p not in sys.path:
        sys.path.insert(0, _p)

import ml_dtypes

import concourse.bacc as bacc
import concourse.tile as tile
from concourse import mybir
from concourse.bass_utils import run_bass_kernel_spmd
from concourse.masks import make_identity

B, N, M, D = 4, 4096, 4096, 64
H = N // 2  # rows per core per side
NCORES = 8
C0 = 40.0
KS = D + 3  # score matmul contraction: features + shift row + 2 bias rows
MB = 128  # m block (score partition dim)
NB = 512  # matmul free dim (PSUM bank)
NMB = M // MB  # 32 m blocks
NNB = H // NB  # 4 n blocks per core
VW = 256  # padded width of the v tensor (128-wide lhsT slices)
F32 = mybir.dt.float32
F16 = mybir.dt.float16
BF16 = mybir.dt.bfloat16
NPBF = ml_dtypes.bfloat16

_PROG = None


LCH = 8  # lhs DMA column chunks
LCW = M // LCH  # 512 columns per chunk


def _load_side(nc, big, names, slow_ring=None):
    """Allocate a side's SBUF tiles and issue its input DMAs in
    compute-consumption order on the sync HWDGE ring (the only fast ring in
    this runtime, ~23GB/s, FIFO). `slow_ring` diverts the last lhs chunks to
    the gpsimd SWDGE ring — they are consumed last, and the trickle-rate
    SWDGE ring still beats queueing them behind everything else."""
    side = names["side"]
    lhs_ch = [
        big.tile([KS, LCW], F16, tag=f"lhs{c}{side}", name=f"lhs{c}")
        for c in range(LCH)
    ]
    rhs = big.tile([KS, H], F16, tag=f"rhs{side}", name="rhs")
    v_sb = big.tile([MB, VW], F16, tag=f"v{side}", name="v")

    def lhs_dma(c, ring):
        ring.dma_start(out=lhs_ch[c], in_=names["lhs"][:, c * LCW : (c + 1) * LCW])

    def rhs_dma(q):
        nc.sync.dma_start(
            out=rhs[:, q * NB : (q + 1) * NB],
            in_=names["rhs"][:, q * NB : (q + 1) * NB],
        )

    n_slow = 3 if slow_ring is not None else 0
    rhs_dma(0)
    lhs_dma(0, nc.sync)
    nc.sync.dma_start(out=v_sb, in_=names["v"])
    if slow_ring is not None:
        for c in range(LCH - n_slow, LCH):
            lhs_dma(c, slow_ring)
    for c in range(1, LCH - n_slow):
        lhs_dma(c, nc.sync)
    for q in range(1, NNB):
        rhs_dma(q)

    def lhs_slice(mi):
        c, o = divmod(mi * MB, LCW)
        return lhs_ch[c][:, o : o + MB]

    return lhs_slice, rhs, v_sb


def _build_side(nc, pools, identity, loaded, out_d, ring):
    big, upool, spool, wpool, epool = pools
    lhs_slice, rhs, v_sb = loaded

    for nj in range(NNB):
        w = wpool.tile([MB, NB], F32, tag="w", name="w")
        ncol = slice(nj * NB, (nj + 1) * NB)
        for mp in range(NMB // 2):  # mi pairs
            s = spool.tile([MB, 2 * NB], F32, tag="s", name="s")
            u = upool.tile([MB, 2 * NB], BF16, tag="u", name="u")
            for half in range(2):
                mi = 2 * mp + half
                nc.tensor.matmul(
                    s[:, half * NB : (half + 1) * NB],
                    lhsT=lhs_slice(mi),
                    rhs=rhs[:, ncol],
                    start=True,
                    stop=True,
                )
            nc.scalar.activation(
                out=u, in_=s, func=mybir.ActivationFunctionType.Exp, scale=2.0
            )
            for half in range(2):
                mi = 2 * mp + half
                nc.tensor.matmul(
                    w,
                    lhsT=v_sb[:, mi * 4 : mi * 4 + MB],
                    rhs=u[:, half * NB : (half + 1) * NB],
                    start=(mi == 0),
                    stop=(mi == NMB - 1),
                )
        # epilogue for this n block: W[0:4,:] = [num_xyz; denom] -> out rows
        w_sb = epool.tile([4, NB], F32, tag="wsb", name="wsb")
        nc.scalar.copy(w_sb, w[0:4, :])
        for j2 in range(NB // MB):
            wt_ps = spool.tile([MB, 4], F32, tag="s", name="wt")
            nc.tensor.transpose(
                wt_ps, w_sb[:, j2 * MB : (j2 + 1) * MB], identity[:4, :4]
            )
            wt_sb = epool.tile([MB, 4], F32, tag="wtsb", name="wtsb")
            nc.vector.tensor_copy(wt_sb, wt_ps)
            r_sb = epool.tile([MB, 1], F32, tag="r", name="r")
            nc.vector.reciprocal(r_sb, wt_sb[:, 3:4])
            o_sb = epool.tile([MB, 3], F32, tag="o", name="o")
            nc.vector.tensor_scalar_mul(o_sb, wt_sb[:, 0:3], r_sb)
            row = nj * NB + j2 * MB
            ring.dma_start(out=out_d[row : row + MB, :], in_=o_sb)


def _build():
    nc = bacc.Bacc("TRN2", target_bir_lowering=False, debug=False)

    sides = []
    for side in ("A", "B"):
        sides.append(
            {
                "side": side,
                "lhs": nc.dram_tensor(
                    f"lhs{side}", [KS, M], F16, kind="ExternalInput"
                ).ap(),
                "rhs": nc.dram_tensor(
                    f"rhs{side}", [KS, H], F16, kind="ExternalInput"
                ).ap(),
                "v": nc.dram_tensor(
                    f"v{side}", [MB, VW], F16, kind="ExternalInput"
                ).ap(),
            }
        )
    out_src = nc.dram_tensor("out_src", [H, 3], F32, kind="ExternalOutput").ap()
    out_tgt = nc.dram_tensor("out_tgt", [H, 3], F32, kind="ExternalOutput").ap()

    with tile.TileContext(nc) as tc:
        with (
            tc.tile_pool(name="big", bufs=2) as big,
            tc.tile_pool(name="upool", bufs=6) as upool,
            tc.tile_pool(name="spool", bufs=3, space="PSUM") as spool,
            tc.tile_pool(name="wpool", bufs=2, space="PSUM") as wpool,
            tc.tile_pool(name="epool", bufs=2) as epool,
            tc.tile_pool(name="ident", bufs=1) as ident,
        ):
            identity = ident.tile([MB, MB], F32, tag="identity", name="identity")
            make_identity(nc, identity[:])
            pools = (big, upool, spool, wpool, epool)
            ldA = _load_side(nc, big, sides[0])
            ldB = _load_side(nc, big, sides[1], slow_ring=nc.gpsimd)
            _build_side(nc, pools, identity, ldA, out_src, nc.sync)
            _build_side(nc, pools, identity, ldB, out_tgt, nc.gpsimd)

    nc.compile()
    return nc


def _hi_lo16(x):
    hi = x.astype(np.float16)
    lo = (x - hi.astype(np.float32)).astype(np.float16)
    return hi, lo


def _prep_inputs(src, tgt, src_f, tgt_f):
    """Build the 8 per-core input maps (host-side sharding + layout prep)."""
    src = np.ascontiguousarray(src, dtype=np.float32)
    tgt = np.ascontiguousarray(tgt, dtype=np.float32)
    src_f = np.ascontiguousarray(src_f, dtype=np.float32)
    tgt_f = np.ascontiguousarray(tgt_f, dtype=np.float32)
    aa = (src_f * src_f).sum(axis=1)  # [B, N]
    bb = (tgt_f * tgt_f).sum(axis=1)  # [B, M]

    def chunk_v(pts):  # [L, 3] -> [MB, VW] bf16, col 4*c+f = [pts|1][c*MB+p, f]
        v = np.concatenate([pts, np.ones((pts.shape[0], 1), np.float32)], axis=1)
        flat = v.reshape(-1, MB, 4).transpose(1, 0, 2).reshape(MB, -1)
        out = np.zeros((MB, VW), np.float32)
        out[:, : flat.shape[1]] = flat
        return np.ascontiguousarray(out.astype(np.float16))

    def side(L, R, bias_m, shift_n, vpts, sl):
        ones_m = np.ones((1, L.shape[1]), np.float16)
        ones_n = np.ones((1, H), np.float16)
        bh, bl = _hi_lo16((C0 - bias_m) * 0.5)
        shift = (-0.5 * shift_n[sl]).astype(np.float16)
        return {
            "lhs": np.ascontiguousarray(
                np.vstack([L.astype(np.float16), ones_m, bh[None, :], bl[None, :]])
            ),
            "rhs": np.ascontiguousarray(
                np.vstack([R[:, sl].astype(np.float16), shift[None, :], ones_n, ones_n])
            ),
            "v": chunk_v(vpts),
        }

    in_maps = []
    for c in range(NCORES):
        b, h = divmod(c, 2)
        sl = slice(h * H, (h + 1) * H)
        A = side(tgt_f[b], src_f[b], bb[b], aa[b], tgt[b], sl)
        Bs = side(src_f[b], tgt_f[b], aa[b], bb[b], src[b], sl)
        m = {k + "A": v for k, v in A.items()}
        m.update({k + "B": v for k, v in Bs.items()})
        in_maps.append(m)
    return in_maps


def run(inputs, trace=False, **kw):
    global _PROG
    if _PROG is None:
        _PROG = _build()
    in_maps = _prep_inputs(
        inputs["src"], inputs["tgt"], inputs["src_f"], inputs["tgt_f"]
    )
    bkr = run_bass_kernel_spmd(
        _PROG, in_maps, core_ids=list(range(NCORES)), trace=trace, **kw
    )
    src_corr = np.zeros((B, N, 3), np.float32)
    tgt_corr = np.zeros((B, M, 3), np.float32)
    for c in range(NCORES):
        b, h = divmod(c, 2)
        sl = slice(h * H, (h + 1) * H)
        src_corr[b, sl] = bkr.results[c]["out_src"]
        tgt_corr[b, sl] = bkr.results[c]["out_tgt"]
    return (src_corr, tgt_corr), bkr


def kernel(**inputs):
    out, _ = run(inputs)
    return out
